# revision 46
# baseline (speedup 1.0000x reference)
"""Distributed Trainium2 Bass kernel for the causal AttentionBlock problem.

Shapes (hardcoded): B=2, S=2048, D=2048, H=16, HD=128, softcap 50, RoPE.

Sharding: DP over batch (2 groups of 4 cores) x TP over heads
(4 heads/core) for QKV+attention; the output projection is M-SHARDED:
after attention, each core's normalized per-head outputs x_h [HD, SC]
are AllGathered across its 4-core group (one batched AG per q-chunk
for chunks 0-2; per-head AGs for chunk 3 so the epilogue pipelines),
and every core computes out[:, m-slice of 512] with the full 16-head
contraction locally - no ReduceScatter, no output reduction.

Key design points (see git-less history in comments):
- softmax row-sums run off the PE: DVE accumulates exp tiles (emitted
  a few units late so they never head-of-line-block RoPE ops in the
  DVE queue) and a Pool-engine partition_all_reduce broadcasts the
  per-q sums; the old ones-matmul row-sums (160 matmuls) are gone.
- causal diagonal k-tiles are extent-trimmed: scores/AV matmuls and
  the exp only cover columns >= the causal extent, and masking is one
  small [128,128] triangular multiply per boundary block.
- emission order IS the schedule (per-engine queues are in-order):
  projections, attention, and output-projection units are woven so
  the ACT-bound tanh/exp stream of the last chunk overlaps all the
  deferred output projections; the t=3 epilogue accumulates each
  AllGather's head-group into 4 PSUM banks (pop+2xppj+pso) so only
  one 16-matmul wave trails the final AG.
- PSUM: psg 2x[128,1024] scores, ppj 2x[128,512] projections,
  ps_o + pop single banks.
- startup: per-do xt0 pieces and per-head wq/wk pieces across the
  sync/scalar/SWDGE queues so the first QK unit starts ~10us in.

Device-side compute dtype: bf16 matmul inputs / fp32 accumulation.
Host-pre-transposed inputs so every matmul contraction is on the
partition axis:
  xt   [4, 128, 16*512]  xt[t,di,do*512+s] = x[t*512+s, do*128+di]
  wq/wk [128, 4, 16, 128] head-major per-core q/k weights
  wv   [128, 16, 512]    wv[di,do,h*128+f] = w_in[do*128+di, h, f+256]
  wo   [128, 16, 512]    wo[hd,hg,m] = w_out[hg, hd, pos*512+m]
  cs2/sn2 [128, 2048]    doubled rope tables (bf16)
  tri  [128, 128]        triangular boundary mask tri[p,c] = (c >= p)
"""

import math
import os
import sys

import numpy as np

sys.path.insert(0, "/opt/trn_rl_repo")

import ml_dtypes  # noqa: E402

import concourse.bass as bass  # noqa: E402
import concourse.bass_isa as bass_isa  # noqa: E402
import concourse.mybir as mybir  # noqa: E402
import concourse.tile as tile  # noqa: E402
from concourse import bacc, bass_utils  # noqa: E402

B, S, D, H = 2, 2048, 2048, 16
HD = 128
HALF = 64
NH = 4          # heads per core
NCORES = 8
GROUPS = [[0, 1, 2, 3], [4, 5, 6, 7]]
SC = 512        # s-chunk (q-chunk size)
MS = 512        # m-slice per core (D / 4)
P = 128
DO = D // P     # 16 d-outer chunks
ST = S // P     # 16 s-tiles
QC = S // SC    # 4 q chunks
SCAP = 50.0
F32 = mybir.dt.float32
BF16 = mybir.dt.bfloat16

_CACHE = {}
LAST_EXEC_NS = None
LAST_RESULT = None


def _install_profile_shim():
    """Provide antenv.axon_hooks (missing in this image) so
    run_bass_kernel_spmd(trace=True) can NTFF-profile via libaxon."""
    import contextlib
    import ctypes
    import types

    try:
        import antenv
    except ImportError:
        return
    if "antenv.axon_hooks" in sys.modules:
        return
    try:
        from antenv import axon_hooks  # noqa: F401
        return
    except ImportError:
        pass
    so_path = "/opt/axon/libaxon_pjrt.so"
    if not os.path.exists(so_path):
        return
    mod = types.ModuleType("antenv.axon_hooks")
    state = {"hook": None}
    mod.set_axon_ntff_profile_hook = lambda h: state.__setitem__("hook", h)
    mod.get_axon_ntff_profile_hook = lambda: state["hook"]
    sys.modules["antenv.axon_hooks"] = mod
    antenv.axon_hooks = mod

    lib = ctypes.CDLL(so_path)
    if not hasattr(lib, "axon_start_nrt_profile"):
        return
    lib.axon_start_nrt_profile.argtypes = [
        ctypes.POINTER(ctypes.c_int64), ctypes.c_size_t]
    lib.axon_start_nrt_profile.restype = ctypes.c_int64
    lib.axon_stop_nrt_profile.argtypes = [ctypes.c_char_p]
    lib.axon_stop_nrt_profile.restype = ctypes.c_int64

    @contextlib.contextmanager
    def _hook(output_dir, device_ids):
        import jax
        jax.devices()
        if device_ids:
            ids = (ctypes.c_int64 * len(device_ids))(*device_ids)
            rc = lib.axon_start_nrt_profile(ids, len(device_ids))
        else:
            rc = lib.axon_start_nrt_profile(None, 0)
        if rc != 0:
            raise RuntimeError(f"axon_start_nrt_profile rc={rc}")
        try:
            yield
        finally:
            n = lib.axon_stop_nrt_profile(str(output_dir).encode())
            print(f"profile: {n} file(s) written to {output_dir}",
                  file=sys.stderr)

    mod.set_axon_ntff_profile_hook(_hook)


def _weave(*seqs):
    """Proportionally interleave several lists."""
    import heapq
    seqs = [s for s in seqs if s]
    h = [(0.5 / len(s), i, 0) for i, s in enumerate(seqs)]
    heapq.heapify(h)
    out = []
    while h:
        _, i, j = heapq.heappop(h)
        out.append(seqs[i][j])
        if j + 1 < len(seqs[i]):
            heapq.heappush(h, ((j + 1.5) / len(seqs[i]), i, j + 1))
    return out


def build_nc():
    nc = bacc.Bacc("TRN2", target_bir_lowering=False, debug=False,
                   num_devices=NCORES)

    xt_d = nc.dram_tensor("xt", [QC, P, DO * SC], BF16, kind="ExternalInput")
    wq_d = nc.dram_tensor("wq", [P, NH, DO, HD], BF16, kind="ExternalInput")
    wk_d = nc.dram_tensor("wk", [P, NH, DO, HD], BF16, kind="ExternalInput")
    wv_d = nc.dram_tensor("wv", [P, DO, NH * HD], BF16, kind="ExternalInput")
    wo_d = nc.dram_tensor("wo", [P, H, MS], BF16, kind="ExternalInput")
    cs2_d = nc.dram_tensor("cs2", [P, S], BF16, kind="ExternalInput")
    sn2_d = nc.dram_tensor("sn2", [P, S], BF16, kind="ExternalInput")
    tri_d = nc.dram_tensor("tri", [P, P], BF16, kind="ExternalInput")
    out_d = nc.dram_tensor("out", [S, MS], BF16, kind="ExternalOutput")

    tanh_scale = 1.0 / (SCAP * math.sqrt(HD))
    MUL = mybir.AluOpType.mult
    ADDOP = mybir.AluOpType.add
    BYPASS = mybir.AluOpType.bypass
    TANH = mybir.ActivationFunctionType.Tanh
    EXP = mybir.ActivationFunctionType.Exp

    with tile.TileContext(nc) as tc:
        with (
            tc.tile_pool(name="pers", bufs=1) as pers,
            tc.tile_pool(name="xtp", bufs=2) as xtp,
            tc.tile_pool(name="etp", bufs=6) as etp,
            tc.tile_pool(name="dvp", bufs=1) as dvp,
            tc.tile_pool(name="accp", bufs=2) as accp,
            tc.tile_pool(name="sump", bufs=2) as sump,
            tc.tile_pool(name="rbp", bufs=2) as rbp,
            tc.tile_pool(name="xhp", bufs=2) as xhp,
            tc.tile_pool(name="xap", bufs=1) as xap,
            tc.tile_pool(name="otp", bufs=2) as otp,
            tc.tile_pool(name="psc", bufs=2, space="PSUM") as psc,
            tc.tile_pool(name="pso", bufs=1, space="PSUM") as pso,
            tc.tile_pool(name="ppj", bufs=2, space="PSUM") as ppj,
            tc.tile_pool(name="pop", bufs=1, space="PSUM") as pop,
            tc.tile_pool(name="dram", bufs=1, space="DRAM") as dpool,
        ):
            # ---- persistent SBUF ----
            QT = pers.tile([P, NH, S], BF16, tag="QT")
            KT = pers.tile([P, NH, S], BF16, tag="KT")
            V = pers.tile([P, ST, NH * HD], BF16, tag="V")
            wq_sb = pers.tile([P, NH, DO, HD], BF16, tag="wq")
            wk_sb = pers.tile([P, NH, DO, HD], BF16, tag="wk")
            wv_lo = pers.tile([P, DO // 2, NH * HD], BF16, tag="wv_lo")
            wv_hi = pers.tile([P, DO // 2, NH * HD], BF16, tag="wv_hi")
            wo_sb = pers.tile([P, H, MS], BF16, tag="wo")
            cs2_sb = pers.tile([P, S], BF16, tag="cs2")
            sn2_sb = pers.tile([P, S], BF16, tag="sn2")
            tri_sb = pers.tile([P, P], BF16, tag="tri")

            xt_c = {}
            HVOL = DO // 2 * SC

            def load_xt(t, eng=None):
                x = xtp.tile([P, DO, SC], BF16, tag="xt", name=f"xt{t}")
                (eng or nc.sync).dma_start(
                    x[:].rearrange("p a b -> p (a b)"), xt_d[t])
                xt_c[t] = x

            # startup: P(0) runs its QK units first, paced by per-head
            # wq/wk pieces on the SWDGE queue and per-do xt0 pieces
            # split across the two HWDGE queues; V units follow with wv
            # streaming on scalar behind the xt0 pieces.
            x0 = xtp.tile([P, DO, SC], BF16, tag="xt", name="xt0")
            x0f = x0[:].rearrange("p a b -> p (a b)")
            nc.sync.dma_start(wq_sb[:, 0, :, :], wq_d[:, 0, :, :])
            for do in range(DO):
                eng = nc.sync if do % 2 == 0 else nc.scalar
                eng.dma_start(x0f[:, do * SC:(do + 1) * SC],
                              xt_d[0][:, do * SC:(do + 1) * SC])
            xt_c[0] = x0
            nc.sync.dma_start(wk_sb[:, 0, :, :], wk_d[:, 0, :, :])
            nc.gpsimd.dma_start(wq_sb[:, 1, :, :], wq_d[:, 1, :, :])
            nc.gpsimd.dma_start(wq_sb[:, 2, :, :], wq_d[:, 2, :, :])
            nc.gpsimd.dma_start(wk_sb[:, 1, :, :], wk_d[:, 1, :, :])
            nc.gpsimd.dma_start(wk_sb[:, 2, :, :], wk_d[:, 2, :, :])
            nc.sync.dma_start(cs2_sb[:], cs2_d[:])
            nc.sync.dma_start(sn2_sb[:], sn2_d[:])
            nc.sync.dma_start(wq_sb[:, 3, :, :], wq_d[:, 3, :, :])
            nc.sync.dma_start(wk_sb[:, 3, :, :], wk_d[:, 3, :, :])
            nc.sync.dma_start(tri_sb[:], tri_d[:])
            for q4 in range(4):
                nc.scalar.dma_start(
                    (wv_lo if q4 < 2 else wv_hi)[:, (q4 % 2) * 4:(q4 % 2) * 4 + 4, :],
                    wv_d[:, q4 * 4:(q4 + 1) * 4, :])
            # xt1 split across both queues; wo is deferred to the
            # first weave (not needed until the O units ~300us in)
            x1 = xtp.tile([P, DO, SC], BF16, tag="xt", name="xt1")
            x1f = x1[:].rearrange("p a b -> p (a b)")
            nc.sync.dma_start(x1f[:, 0:HVOL], xt_d[1][:, 0:HVOL])
            nc.scalar.dma_start(x1f[:, HVOL:2 * HVOL],
                                xt_d[1][:, HVOL:2 * HVOL])
            xt_c[1] = x1

            # DRAM scratch for the x exchange. Chunks 0-2 batch all 4
            # heads into ONE AllGather (fired at the last head); chunk 3
            # keeps per-head AGs so the epilogue waves pipeline.
            xag = {}
            xga = {}
            for t in range(QC - 1):
                xag[t] = dpool.tile(
                    [NH * P, SC], BF16, tag=f"xag{t}", name=f"xag{t}")
                xga[t] = dpool.tile(
                    [4 * NH * P, SC], BF16, tag=f"xga{t}", name=f"xga{t}")
            for h in range(NH):
                xag[(3, h)] = dpool.tile(
                    [P, SC], BF16, tag=f"xag3_{h}", name=f"xag3_{h}")
                xga[(3, h)] = dpool.tile(
                    [4 * P, SC], BF16, tag=f"xga3_{h}",
                    name=f"xga3_{h}")

            # ---------------- unit builders ----------------
            def unit_V(t, stl):
                def f(t=t, stl=stl):
                    ps = ppj.tile([P, SC], F32, tag="ppj")
                    xc = xt_c[t]
                    for do in range(DO):
                        xs = xc[:, do, :]
                        wvh = wv_lo if do < DO // 2 else wv_hi
                        wvs = wvh[:, do % (DO // 2), :]
                        nc.tensor.matmul(
                            ps[:],
                            lhsT=xs[:, stl * P:(stl + 1) * P],
                            rhs=wvs,
                            start=(do == 0), stop=(do == DO - 1))
                    nc.scalar.copy(V[:, 4 * t + stl, :], ps[:])
                return f

            def unit_QK(t, h, which):
                def f(t=t, h=h, which=which):
                    w_sb = wq_sb if which == 0 else wk_sb
                    dst = QT if which == 0 else KT
                    ps = ppj.tile([P, SC], F32, tag="ppj")
                    xc = xt_c[t]
                    for do in range(DO):
                        xs = xc[:, do, :]
                        nc.tensor.matmul(
                            ps[:], lhsT=w_sb[:, h, do, :],
                            rhs=xs,
                            start=(do == 0), stop=(do == DO - 1))
                    sl = slice(t * SC, (t + 1) * SC)
                    tcos = dvp.tile([P, SC], F32, tag="tcos")
                    tsin = dvp.tile([P, SC], F32, tag="tsin")
                    nc.vector.tensor_tensor(
                        tcos[:], ps[:], cs2_sb[:, sl], MUL)
                    nc.vector.tensor_tensor(
                        tsin[0:HALF, :], ps[HALF:P, :],
                        sn2_sb[0:HALF, sl], MUL)
                    nc.vector.tensor_tensor(
                        tsin[HALF:P, :], ps[0:HALF, :],
                        sn2_sb[HALF:P, sl], MUL)
                    nc.vector.tensor_tensor(
                        dst[:, h, sl], tcos[:], tsin[:], ADDOP)
                return f

            def P_units(t):
                units = []
                for h in range(NH):
                    units.append(unit_QK(t, h, 0))
                    units.append(unit_QK(t, h, 1))
                vu = [unit_V(t, stl) for stl in range(4)]
                if t == 0:
                    units.extend(vu)       # wq/wk stream in first
                else:
                    units = vu + units
                return units

            # ---- attention ----
            # Causal extent of k-tile kc for q-chunk t: columns below
            # ext are entirely masked (skipped); the first 128 columns
            # at ext are the triangular boundary (tri mask).
            def ext(t, kc):
                return max(0, (kc - 4 * t) * P)

            def flush_add(st, t, g, et):
                """Row-sum accumulation on DVE, deferred a few units so
                the exp dependency is long done and these never
                head-of-line-block rope ops in the DVE queue."""
                e0 = ext(t, 2 * g)
                e1 = ext(t, 2 * g + 1)
                acc = st["acc_a"]
                if g == 0:
                    if e1 == 0:
                        nc.vector.tensor_tensor(
                            acc[:], et[:, 0:SC], et[:, SC:2 * SC], ADDOP)
                    else:
                        nc.vector.tensor_copy(acc[:], et[:, 0:SC])
                        nc.vector.tensor_tensor(
                            acc[:, e1:SC], acc[:, e1:SC],
                            et[:, SC + e1:2 * SC], ADDOP)
                else:
                    nc.vector.tensor_tensor(
                        acc[:, e0:SC], acc[:, e0:SC], et[:, e0:SC], ADDOP)
                    nc.vector.tensor_tensor(
                        acc[:, e1:SC], acc[:, e1:SC],
                        et[:, SC + e1:2 * SC], ADDOP)

            def consume(t, h, st):
                """Mask then AV matmuls for the previous score pair;
                queue its row-sum adds for deferred emission."""
                g, et = st.pop("prev")
                nkc = 4 * (t + 1)
                if st.get("ps_o") is None:
                    st["ps_o"] = pso.tile([P, SC], F32, tag="pso",
                                          name=f"pso{t}_{h}")
                    st["acc_a"] = accp.tile([P, SC], F32, tag="acc_a",
                                            name=f"acca{t}_{h}")
                ps_o = st["ps_o"]
                for j in (0, 1):
                    kc = 2 * g + j
                    e = ext(t, kc)
                    if kc >= 4 * t:  # triangular boundary block
                        nc.vector.tensor_tensor(
                            et[:, j * SC + e:j * SC + e + P],
                            et[:, j * SC + e:j * SC + e + P],
                            tri_sb[:], MUL)
                    nc.tensor.matmul(
                        ps_o[:, e:SC],
                        lhsT=V[:, kc, h * HD:(h + 1) * HD],
                        rhs=et[:, j * SC + e:(j + 1) * SC],
                        start=(kc == 0), stop=(kc == nkc - 1))
                pend = st.setdefault("pend_add", [])
                pend.append((g, et))
                while len(pend) > 2:
                    ga, eta = pend.pop(0)
                    flush_add(st, t, ga, eta)

            def unit_A(t, h, g, st):
                def f(t=t, h=h, g=g, st=st):
                    psg = psc.tile([P, 2 * SC], F32, tag="score")
                    for j in (0, 1):
                        kc = 2 * g + j
                        e = ext(t, kc)
                        nc.tensor.matmul(
                            psg[:, j * SC + e:(j + 1) * SC],
                            lhsT=KT[:, h, kc * P:(kc + 1) * P],
                            rhs=QT[:, h, t * SC + e:(t + 1) * SC],
                            start=True, stop=True)
                    et = etp.tile([P, 2 * SC], BF16, tag="et")
                    if g == 2 * t + 1:
                        # second diagonal pair: only columns >=256 of
                        # each half are live; one strided rectangle op
                        psg3 = psg[:].rearrange(
                            "p (j q) -> p j q", j=2)[:, :, SC // 2:SC]
                        et3 = et[:].rearrange(
                            "p (j q) -> p j q", j=2)[:, :, SC // 2:SC]
                        nc.scalar.activation(psg3, psg3, TANH,
                                             scale=tanh_scale)
                        nc.scalar.activation(et3, psg3, EXP, scale=SCAP)
                    else:
                        nc.scalar.activation(psg[:], psg[:], TANH,
                                             scale=tanh_scale)
                        nc.scalar.activation(et[:], psg[:], EXP,
                                             scale=SCAP)
                    if "prev" in st:
                        consume(t, h, st)
                    st["prev"] = (g, et)
                return f

            def unit_Afin1(t, h, st):
                def f(t=t, h=h, st=st):
                    consume(t, h, st)
                    for ga, eta in st.pop("pend_add"):
                        flush_add(st, t, ga, eta)
                    sums = sump.tile([P, SC], F32, tag="sums",
                                     name=f"sums{t}_{h}")
                    nc.gpsimd.partition_all_reduce(
                        sums[:], st["acc_a"][:], channels=P,
                        reduce_op=bass_isa.ReduceOp.add)
                    st["sums"] = sums
                return f

            def unit_Afin2(t, h, st):
                def f(t=t, h=h, st=st):
                    rb = rbp.tile([P, SC], F32, tag="rb")
                    nc.vector.reciprocal_approx_fast(
                        out=rb[:], in_=st["sums"][:])
                    xh = xhp.tile([P, SC], BF16, tag="xh")
                    nc.vector.tensor_tensor(
                        xh[:], st["ps_o"], rb[:], MUL)
                    if t == QC - 1:
                        nc.sync.dma_start(xag[(t, h)][:], xh[:])
                        nc.gpsimd.collective_compute(
                            "AllGather", BYPASS,
                            replica_groups=GROUPS,
                            ins=[xag[(t, h)].opt()],
                            outs=[xga[(t, h)].opt()],
                        )
                    else:
                        nc.sync.dma_start(
                            xag[t][h * P:(h + 1) * P, :], xh[:])
                        if h == NH - 1:
                            nc.gpsimd.collective_compute(
                                "AllGather", BYPASS,
                                replica_groups=GROUPS,
                                ins=[xag[t].opt()],
                                outs=[xga[t].opt()],
                            )
                return f

            def A_units(t, hs=None):
                units = []
                pend = None
                for h in (hs if hs is not None else range(NH)):
                    st = {}
                    for i, g in enumerate(range(2 * (t + 1))):
                        units.append(unit_A(t, h, g, st))
                        if i == 0 and pend is not None:
                            units.append(pend)
                            pend = None
                    units.append(unit_Afin1(t, h, st))
                    pend = unit_Afin2(t, h, st)
                units.append(pend)
                return units

            # ---- readback + output projection ----
            xall_c = {}

            def unit_RB(t):
                def f(t=t):
                    # xall[p, h, s, q] = x of global head 4*s+h
                    xall = xap.tile([P, NH, 4, SC], BF16, tag="xall",
                                    name=f"xall{t}")
                    for h in range(NH):
                        for s4 in range(4):
                            nc.sync.dma_start(
                                xall[:, h, s4, :],
                                xga[t][s4 * NH * P + h * P:
                                       s4 * NH * P + (h + 1) * P, :])
                    xall_c[t] = xall
                return f

            def unit_O(t, u):
                def f(t=t, u=u):
                    xall = xall_c[t]
                    ps = pop.tile([P, MS], F32, tag="pop")
                    for hg in range(H):
                        nc.tensor.matmul(
                            ps[:],
                            lhsT=xall[:, hg % NH, hg // NH,
                                      u * P:(u + 1) * P],
                            rhs=wo_sb[:, hg, :],
                            start=(hg == 0), stop=(hg == H - 1))
                    ot = otp.tile([P, MS], BF16, tag="ot")
                    nc.vector.tensor_copy(ot[:], ps[:])
                    nc.sync.dma_start(
                        out_d[t * SC + u * P: t * SC + (u + 1) * P, :],
                        ot[:])
                return f

            def O_units(t):
                return [unit_O(t, u) for u in range(4)]

            # ---------------- emission pipeline ----------------
            # t=0: emit just h0's QK, the V units, and h1's QK, then
            # weave A(0) with the rest of P(0) and P(1) - A(0,h0) only
            # needs h0's Q/K + V, so the PE gets attention fill work
            # while the startup DMAs still stream.
            p0 = P_units(0)   # [qk h0..h3 (8 units), V0..V3 (4 units)]
            for un in p0[0:2] + p0[8:12] + p0[2:4]:
                un()
            load_xt(2, nc.scalar)
            p1 = P_units(1)
            for un in _weave(A_units(0), p0[4:8] + p1):
                un()
            load_xt(3, nc.scalar)
            nc.scalar.dma_start(wo_sb[:], wo_d[:])
            for un in _weave(A_units(1), P_units(2)):
                un()
            for un in _weave(A_units(2), P_units(3)):
                un()
            # A(3) is ACT-heavy (tanh/exp); fill the PE with O units.
            # O(2) goes after the weave: it fills the AG(3,h) waits
            # right before the wave epilogue.
            late = ([unit_RB(0)] + O_units(0) + [unit_RB(1)] + O_units(1))
            for un in _weave(A_units(3), late):
                un()
            unit_RB(2)()
            for un in O_units(2):
                un()
            # t=3 epilogue: wave-structured output projection. Per
            # AllGather(3,h) completion, accumulate that head-group's
            # contribution for qsubs 0-2 (banks: pop + 2x ppj, which the
            # finished P units freed); qsub 3 uses the pso bank (freed
            # by Afin2(3,h3)) and runs its h0-h2 waves during the last
            # AG wait, so only one 16-matmul wave trails the final AG.
            xall3 = xap.tile([P, NH, 4, SC], BF16, tag="xall",
                             name="xall3")
            qacc = [pop.tile([P, MS], F32, tag="pop", name="w3q0"),
                    ppj.tile([P, MS], F32, tag="ppj", name="w3q1"),
                    ppj.tile([P, MS], F32, tag="ppj", name="w3q2")]

            def wave(h, us, qacc=qacc):
                for s4 in range(4):
                    nc.sync.dma_start(
                        xall3[:, h, s4, :],
                        xga[(3, h)][s4 * P:(s4 + 1) * P, :])
                for u in us:
                    for s in range(4):
                        nc.tensor.matmul(
                            qacc[u][:],
                            lhsT=xall3[:, h, s, u * P:(u + 1) * P],
                            rhs=wo_sb[:, 4 * s + h, :],
                            start=(h == 0 and s == 0),
                            stop=(h == NH - 1 and s == 3))

            for h in range(3):
                wave(h, (0, 1, 2))
            q3 = pso.tile([P, MS], F32, tag="pso", name="w3q3")
            qacc.append(q3)
            for h in range(3):
                for s in range(4):
                    nc.tensor.matmul(
                        q3[:],
                        lhsT=xall3[:, h, s, 3 * P:4 * P],
                        rhs=wo_sb[:, 4 * s + h, :],
                        start=(h == 0 and s == 0), stop=False)
            for s4 in range(4):
                nc.sync.dma_start(
                    xall3[:, 3, s4, :],
                    xga[(3, 3)][s4 * P:(s4 + 1) * P, :])
            for u in range(4):
                for s in range(4):
                    nc.tensor.matmul(
                        qacc[u][:],
                        lhsT=xall3[:, 3, s, u * P:(u + 1) * P],
                        rhs=wo_sb[:, 4 * s + 3, :],
                        start=False, stop=(s == 3))
                ot = otp.tile([P, MS], BF16, tag="ot")
                nc.vector.tensor_copy(ot[:], qacc[u][:])
                (nc.sync if u % 2 == 0 else nc.scalar).dma_start(
                    out_d[3 * SC + u * P: 3 * SC + (u + 1) * P, :], ot[:])

    nc.compile()
    return nc


def _prep_core_inputs(inputs, w_in, w_out, rope_sin, rope_cos):
    """Build the 8 per-core input maps (numpy, pre-transposed, bf16)."""
    bf = ml_dtypes.bfloat16
    cs2 = np.concatenate([rope_cos.T, rope_cos.T], axis=0).astype(bf)
    sn2 = np.concatenate([-rope_sin.T, rope_sin.T], axis=0).astype(bf)
    cs2 = np.ascontiguousarray(cs2)
    sn2 = np.ascontiguousarray(sn2)
    # triangular boundary mask: tri[p, c] = (c >= p)
    tri = np.ascontiguousarray(
        (np.arange(P)[None, :] >= np.arange(P)[:, None]).astype(bf))
    in_maps = []
    for c in range(NCORES):
        g, pos = c // 4, c % 4
        hsel = slice(4 * pos, 4 * pos + 4)
        xt = np.ascontiguousarray(
            inputs[g].T.reshape(DO, P, QC, SC).transpose(2, 1, 0, 3)
            .reshape(QC, P, DO * SC)).astype(bf)
        wq = np.ascontiguousarray(
            w_in[:, hsel, 0:HD].reshape(DO, P, NH, HD).transpose(1, 2, 0, 3)
        ).astype(bf)
        wk = np.ascontiguousarray(
            w_in[:, hsel, HD:2 * HD].reshape(DO, P, NH, HD)
            .transpose(1, 2, 0, 3)).astype(bf)
        wv = np.ascontiguousarray(
            w_in[:, hsel, 2 * HD:3 * HD].reshape(DO, P, NH, HD)
            .transpose(1, 0, 2, 3).reshape(P, DO, NH * HD)).astype(bf)
        wo = np.ascontiguousarray(
            w_out[:, :, pos * MS:(pos + 1) * MS].transpose(1, 0, 2)
        ).astype(bf)
        in_maps.append({"xt": xt, "wq": wq, "wk": wk, "wv": wv, "wo": wo,
                       "cs2": cs2, "sn2": sn2, "tri": tri})
    return in_maps


def kernel(inputs, w_in, w_out, rope_sin, rope_cos, mask=None):
    global LAST_EXEC_NS, LAST_RESULT
    inputs = np.asarray(inputs, dtype=np.float32)
    w_in = np.asarray(w_in, dtype=np.float32)
    w_out = np.asarray(w_out, dtype=np.float32)
    rope_sin = np.asarray(rope_sin, dtype=np.float32)
    rope_cos = np.asarray(rope_cos, dtype=np.float32)

    if "nc" not in _CACHE:
        _CACHE["nc"] = build_nc()
    nc = _CACHE["nc"]

    in_maps = _prep_core_inputs(inputs, w_in, w_out, rope_sin, rope_cos)
    trace = bool(int(os.environ.get("BASS_PROFILE", "0")))
    if trace:
        _install_profile_shim()
    tmpdir = os.environ.get("BASS_TRACE_DIR") or None
    try:
        res = bass_utils.run_bass_kernel_spmd(
            nc, in_maps, core_ids=list(range(NCORES)), trace=trace,
            tmpdir=tmpdir)
    except Exception:
        if not trace:
            raise
        res = bass_utils.run_bass_kernel_spmd(
            nc, in_maps, core_ids=list(range(NCORES)), trace=False)
    LAST_EXEC_NS = res.exec_time_ns
    LAST_RESULT = res

    out = np.empty((B, S, D), dtype=np.float32)
    for c in range(NCORES):
        g, pos = c // 4, c % 4
        o = np.asarray(res.results[c]["out"]).astype(np.float32)
        out[g, :, pos * MS:(pos + 1) * MS] = o
    return out
